# revision 39
# baseline (speedup 1.0000x reference)
"""GPT2 eager causal attention (B=2, S=2048, D=1024, H=16, HD=64) on 8 TRN2 NeuronCores.

Sharding (data + head/tensor parallel): core c -> (batch b = c//4, head-group
g = c%4), 4 heads per group.  Token ownership for the output: core (b, g) owns
token rows [ch*512 + g*128, +128) of batch b, for each 512-token chunk ch.

v2 pipeline (vs the RS baseline):
  - x is transposed on the HOST -> xT [D, S]; no transpose-DMAs on device.
  - weights are host-packed into SBUF layout -> one large DMA each, spread
    across the SP/ACT/gpsimd DMA queues so nothing serializes at startup.
  - QT/KT [256, S] and V [S, 260] (ones-column per head for the free softmax
    denominator) as in the baseline, but QKV compute for token-chunk sq>=1 is
    software-pipelined INTO the attention loop of chunk sq-1 to keep PE dense
    (HAM stays warm) and to overlap ACT-exp with PE matmuls.
  - scores: the two heads of a KT row-tile run as CONCURRENT 64-contract
    matmuls on PE row-groups 0-63/64-127 writing adjacent PSUM banks; one
    batched EXP (FD=1024) covers both.  Diagonal tiles exp the full tile
    (garbage prefix cols are simply never streamed by the AV matmul) and
    triangular-mask only the 128-wide diagonal block.
  - softmax normalize: DVE reciprocal of the ones-row directly from PSUM,
    gpsimd partition_broadcast (instead of a PE rank-1 matmul), DVE multiply.
  - c_proj: per 512-token chunk, AllToAll over the quad exchanges bf16 head
    outputs so each core receives ALL 1024 channels for ITS OWN 128-token
    slice; c_proj then contracts the full D with the full w_proj.  ~4x less
    wire than the fp32 ReduceScatter and a much shorter serial tail.
"""
from collections import deque
from contextlib import ExitStack

import ml_dtypes
import numpy as np

import concourse.bacc as bacc
import concourse.mybir as mybir
import concourse.tile as tile
from concourse.bass import ds as bass_ds
from concourse.bass_utils import run_bass_kernel_spmd

F32 = mybir.dt.float32
BF16 = mybir.dt.bfloat16
U16 = mybir.dt.uint16

B, S, D, H, HD = 2, 2048, 1024, 16, 64
N_CORES = 8
HG = 4               # heads per group
DG = HG * HD         # 256 q/k channels per group
VW = HG * (HD + 1)   # 260: 64 v-cols + 1 ones-col per head
NK = D // 128        # 8 contraction tiles over d
NS = S // 128        # 16 token tiles
CH = 512             # q-chunk (one PSUM bank of fp32)
NCH = S // CH        # 4
NRT = DG // 128      # 2 channel row-tiles (head pairs) per group
SQW = NK * CH        # 4096: xT sbuf columns per token chunk

EXP = mybir.ActivationFunctionType.Exp
IDENT = mybir.ActivationFunctionType.Identity


def _build(has_bv: bool, has_bp: bool, has_bqk: bool):
    nc = bacc.Bacc("TRN2", target_bir_lowering=False, debug=False, num_devices=N_CORES)

    xT_d = nc.dram_tensor("xT", [128, NK * S], BF16, kind="ExternalInput").ap()
    # wq | wk | wv | mask packed back-to-back: one startup DMA
    WQKV = 2 * NK * DG + NK * VW + 128
    wqkv_d = nc.dram_tensor("wqkv", [128, WQKV], BF16, kind="ExternalInput").ap()
    wp_d = nc.dram_tensor("wp", [128, NK * D], BF16, kind="ExternalInput").ap()
    wl_d = nc.dram_tensor("wploc", [128, NRT * D], BF16, kind="ExternalInput").ap()
    bq_d = nc.dram_tensor("bq", [DG, 1], F32, kind="ExternalInput").ap()
    bk_d = nc.dram_tensor("bk", [DG, 1], F32, kind="ExternalInput").ap()
    bv_d = nc.dram_tensor("bv", [DG, 1], F32, kind="ExternalInput").ap()
    bp_d = nc.dram_tensor("bp", [128, D], F32, kind="ExternalInput").ap()
    bp3_d = nc.dram_tensor("bp3", [128, D], F32, kind="ExternalInput").ap()
    mk_d = nc.dram_tensor("masks", [128, 128], BF16, kind="ExternalInput").ap()
    gc_d = nc.dram_tensor("gcol", [1, 1], mybir.dt.uint32, kind="ExternalInput").ap()
    out_d = nc.dram_tensor("out", [(NCH - 1) * 128, D], F32, kind="ExternalOutput").ap()
    out3_d = nc.dram_tensor("out3", [128, D], BF16, kind="ExternalOutput").ap()

    with ExitStack() as ctx:
        tc = ctx.enter_context(tile.TileContext(nc))
        wpool = ctx.enter_context(tc.tile_pool(name="w", bufs=1))
        qkvp = ctx.enter_context(tc.tile_pool(name="qkv", bufs=1))
        stp = ctx.enter_context(tc.tile_pool(name="stx", bufs=3))
        nrm = ctx.enter_context(tc.tile_pool(name="nrm", bufs=3))
        otfp = ctx.enter_context(tc.tile_pool(name="otf", bufs=2))
        outp = ctx.enter_context(tc.tile_pool(name="outp", bufs=3))
        ps_st = ctx.enter_context(tc.tile_pool(name="psst", bufs=2, space="PSUM"))
        ps_ot = ctx.enter_context(tc.tile_pool(name="psot", bufs=2, space="PSUM"))
        ps_mm = ctx.enter_context(tc.tile_pool(name="psmm", bufs=2, space="PSUM"))
        dram = ctx.enter_context(tc.tile_pool(name="dram", bufs=1, space="DRAM"))

        # ---- tiny exp at t=0 forces the ACT table load to overlap the DMAs
        dmy = wpool.tile([128, 1], F32, tag="dmy", name="dmy")
        dmo = wpool.tile([128, 1], F32, tag="dmo", name="dmo")
        nc.vector.memset(dmy[:], 0.0)
        nc.scalar.activation(dmo[:], dmy[:], EXP, scale=1.0)

        # ---- weights: wq|wk|wv|mask as ONE contiguous DMA on the ACT queue,
        # wp (only needed ~100us in) on the gpsimd queue
        wqkv_sb = wpool.tile([128, WQKV], BF16, tag="wqkv", name="wqkv_sb")
        wp_sb = wpool.tile([128, NK * D], BF16, tag="wp", name="wp_sb")
        wl_sb = wpool.tile([128, NRT * D], BF16, tag="wl", name="wl_sb")
        nc.scalar.dma_start(wqkv_sb[:], wqkv_d[:])
        nc.gpsimd.dma_start(wp_sb[:], wp_d[:])
        nc.gpsimd.dma_start(wl_sb[:], wl_d[:])
        wq_sb = wqkv_sb[:, 0:NK * DG]
        wk_sb = wqkv_sb[:, NK * DG:2 * NK * DG]
        wv_sb = wqkv_sb[:, 2 * NK * DG:2 * NK * DG + NK * VW]
        mk_sb = wqkv_sb[:, 2 * NK * DG + NK * VW:WQKV]
        bq_sb = bk_sb = bv_sb = bp_sb = None
        if has_bqk:
            bq_sb = wpool.tile([128, NRT], F32, tag="bq", name="bq_sb")
            bk_sb = wpool.tile([128, NRT], F32, tag="bk", name="bk_sb")
            for rt in range(NRT):
                nc.scalar.dma_start(bq_sb[:, rt:rt + 1], bq_d[rt * 128:(rt + 1) * 128, :])
                nc.scalar.dma_start(bk_sb[:, rt:rt + 1], bk_d[rt * 128:(rt + 1) * 128, :])
        if has_bv:
            bv_sb = wpool.tile([128, NRT], F32, tag="bv", name="bv_sb")
            for rt in range(NRT):
                nc.scalar.dma_start(bv_sb[:, rt:rt + 1], bv_d[rt * 128:(rt + 1) * 128, :])
        bp3_sb = None
        if has_bp:
            bp_sb = wpool.tile([128, D], F32, tag="bp", name="bp_sb")
            nc.scalar.dma_start(bp_sb[:], bp_d[:])
            # ch3 partial-sum path: only the g==0 rank may add the bias
            bp3_sb = wpool.tile([128, D], F32, tag="bp3", name="bp3_sb")
            nc.scalar.dma_start(bp3_sb[:], bp3_d[:])

        # ---- V tiles exist up-front; set the whole tile to bf16 1.0 now so
        # the later per-head copies only write the 64 v-columns
        V = []
        for st in range(NS):
            vt = qkvp.tile([128, VW], BF16, tag=f"v{st}", name=f"v{st}")
            nc.vector.memset(vt[:].bitcast(U16), 0x3F80)
            V.append(vt)

        # ---- xT loads: host pre-packed to SBUF layout (col = sq*4096 + kt*512
        # + t), so each chunk is one fully-contiguous 1MB DMA
        xT = qkvp.tile([128, NK * S], BF16, tag="xT", name="xT")
        # sq0 lands in two halves so the first Q chain starts ~2us sooner
        nc.sync.dma_start(xT[:, 0:SQW // 2], xT_d[:, 0:SQW // 2])
        nc.sync.dma_start(xT[:, SQW // 2:SQW], xT_d[:, SQW // 2:SQW])
        for sq in range(1, NCH):
            nc.sync.dma_start(
                xT[:, sq * SQW:(sq + 1) * SQW], xT_d[:, sq * SQW:(sq + 1) * SQW]
            )

        QT = [qkvp.tile([128, S], BF16, tag=f"qt{rt}", name=f"qt{rt}") for rt in range(NRT)]
        KT = [qkvp.tile([128, S], BF16, tag=f"kt{rt}", name=f"kt{rt}") for rt in range(NRT)]
        OT = [qkvp.tile([128, S], BF16, tag=f"ot{rt}", name=f"ot{rt}") for rt in range(NRT)]

        def emit_qk(store, w_sb, b_sb, rt, sq):
            ps = ps_mm.tile([128, CH], F32, tag="ps", name=f"psqk{rt}_{sq}")
            for kt in range(NK):
                nc.tensor.matmul(
                    ps[:],
                    w_sb[:, kt * DG + rt * 128: kt * DG + (rt + 1) * 128],
                    xT[:, sq * SQW + kt * CH: sq * SQW + (kt + 1) * CH],
                    start=(kt == 0), stop=(kt == NK - 1),
                )
            dst = store[rt][:, sq * CH:(sq + 1) * CH]
            if b_sb is not None:
                nc.scalar.activation(dst, ps[:], IDENT, bias=b_sb[:, rt:rt + 1])
            else:
                nc.vector.tensor_copy(dst, ps[:])

        def emit_v(st):
            sq, j = divmod(st, 4)
            ps = ps_mm.tile([128, CH], F32, tag="ps", name=f"psv{st}")
            for kt in range(NK):
                nc.tensor.matmul(
                    ps[:, :VW],
                    xT[:, sq * SQW + kt * CH + j * 128: sq * SQW + kt * CH + (j + 1) * 128],
                    wv_sb[:, kt * VW:(kt + 1) * VW],
                    start=(kt == 0), stop=(kt == NK - 1),
                )
            for hl in range(HG):
                nc.vector.tensor_copy(
                    V[st][:, hl * (HD + 1): hl * (HD + 1) + HD],
                    ps[:, hl * (HD + 1): hl * (HD + 1) + HD],
                )

        # ---- QKV for chunk 0 up-front; chunks 1-3 become fill units that are
        # interleaved into the attention loop (PE never idles on exp waits)
        for rt in range(NRT):
            emit_qk(QT, wq_sb, bq_sb, rt, 0)
            emit_qk(KT, wk_sb, bk_sb, rt, 0)
        for st in range(4):
            emit_v(st)

        fills = {ch: deque() for ch in range(NCH)}
        for sq in range(1, NCH):
            fq = fills[sq - 1]
            fq.append(lambda sq=sq: emit_qk(QT, wq_sb, bq_sb, 0, sq))
            fq.append(lambda sq=sq: emit_qk(KT, wk_sb, bk_sb, 0, sq))
            fq.append(lambda sq=sq: emit_v(sq * 4 + 0))
            fq.append(lambda sq=sq: emit_qk(QT, wq_sb, bq_sb, 1, sq))
            fq.append(lambda sq=sq: emit_qk(KT, wk_sb, bk_sb, 1, sq))
            fq.append(lambda sq=sq: emit_v(sq * 4 + 1))
            fq.append(lambda sq=sq: emit_v(sq * 4 + 2))
            fq.append(lambda sq=sq: emit_v(sq * 4 + 3))

        # ---- collective buffers: per 512-token chunk, AllGather the quad's
        # bf16 head outputs [256 ch, 512 tok] -> [1024 ch, 512 tok]; each core
        # then reads back only ITS OWN 128-token column slice (dynamic offset
        # g*128 from the per-core "gcol" input).
        ag_in = [dram.tile([DG, CH], BF16, tag=f"agi{c}", name=f"ag_in{c}")
                 for c in range(NCH - 1)]
        ag_out = [dram.tile([HG * DG, CH], BF16, tag=f"ago{c}", name=f"ag_out{c}")
                  for c in range(NCH - 1)]
        # chunk 3 goes the other way: local-head c_proj partials, then a
        # bf16 ReduceScatter(add) straight into each core's token slice
        p3 = dram.tile([CH, D], BF16, tag="p3", name="partials3")
        rs3 = dram.tile([128, D], BF16, tag="rs3", name="rs3_out")

        # gc load rides behind the xT loads — it's only needed ~100us in
        gc_sb = wpool.tile([1, 1], mybir.dt.uint32, tag="gc", name="gc_sb")
        nc.scalar.dma_start(gc_sb[:], gc_d[:])

        def emit_ag_in(ch, rt):
            nc.sync.dma_start(
                ag_in[ch][rt * 128:(rt + 1) * 128, :],
                OT[rt][:, ch * CH:(ch + 1) * CH],
            )

        def emit_a2a(ch):
            nc.gpsimd.collective_compute(
                "AllGather",
                mybir.AluOpType.bypass,
                replica_groups=[[0, 1, 2, 3], [4, 5, 6, 7]],
                ins=[ag_in[ch].opt()],
                outs=[ag_out[ch].opt()],
            )

        otf_tiles = {}

        # otf readback rides the gpsimd (SWDGE) queue: emitted right after the
        # NEXT chunk's AG trigger, its wait on AG(ch)-done never heads-of-line
        # blocks the SP queue or the broadcasts
        gcol_gp = None

        def emit_otf(ch):
            nonlocal gcol_gp
            if gcol_gp is None:
                _r = nc.gpsimd.alloc_register("gcol_gp_reg")
                nc.gpsimd.reg_load(_r, gc_sb[0:1, 0:1])
                gcol_gp = nc.gpsimd.snap(_r, donate=True, min_val=0, max_val=(HG - 1) * 128)
            otf = otfp.tile([128, NK * 128], BF16, tag="otf", name=f"otf{ch}")
            src = ag_out[ch][:, bass_ds(gcol_gp, 128)]
            nc.gpsimd.dma_start(
                otf[:].rearrange("p (kt t) -> p kt t", kt=NK),
                src.rearrange("(kt p) t -> p kt t", kt=NK),
            )
            otf_tiles[ch] = otf

        def emit_cproj(ch):
            otf = otf_tiles[ch]
            for n in range(2):
                po = ps_mm.tile([128, CH], F32, tag="ps", name=f"po{ch}_{n}")
                for kt in range(NK):
                    nc.tensor.matmul(
                        po[:],
                        otf[:, kt * 128:(kt + 1) * 128],
                        wp_sb[:, kt * D + n * CH: kt * D + (n + 1) * CH],
                        start=(kt == 0), stop=(kt == NK - 1),
                    )
                ob = outp.tile([128, CH], F32, tag="ob", name=f"ob{ch}_{n}")
                if has_bp:
                    nc.vector.tensor_add(ob[:], po[:], bp_sb[:, n * CH:(n + 1) * CH])
                else:
                    nc.vector.tensor_copy(ob[:], po[:])
                nc.sync.dma_start(out_d[ch * 128:(ch + 1) * 128, n * CH:(n + 1) * CH], ob[:])

        def normalize(ot_ps, rt, off, ch):
            den = nrm.tile([1, CH], F32, tag="den", name=f"den{ch}_{rt}_{off}")
            nc.vector.tensor_copy(den[:], ot_ps[64:65, :])
            rden = nrm.tile([1, CH], F32, tag="rden", name=f"rden{ch}_{rt}_{off}")
            nc.vector.reciprocal_approx_fast(rden[:], den[:])
            rbc = nrm.tile([64, CH], F32, tag="rbc", name=f"rbc{ch}_{rt}_{off}")
            nc.gpsimd.partition_broadcast(rbc[:], rden[:], channels=64)
            dst = OT[rt][off:off + 64, ch * CH:(ch + 1) * CH]
            nc.vector.tensor_mul(dst, ot_ps[0:64, :], rbc[:])
            if has_bv:
                nc.vector.tensor_scalar_add(dst, dst, bv_sb[off:off + 64, rt:rt + 1])

        # ---- attention chunks, with QKV fill units woven in.  cproj for
        # chunk ch-1 (otf readback + matmuls) is emitted right after chunk
        # ch's AllGather trigger: by then AG(ch-1) has completed, so neither
        # the PE FIFO nor the SP DMA queue ever blocks on a collective
        # mid-stream — only the final chunk's AG is exposed.
        for ch in range(NCH):
            nkt = 4 * (ch + 1)
            nsteps = 2 * nkt
            fq = fills[ch]
            n_fill = len(fq)
            emitted = 0
            step = 0
            for rt in range(NRT):
                ot_a = ps_ot.tile([65, CH], F32, tag="ot", name=f"ota{ch}_{rt}")
                ot_b = ps_ot.tile([65, CH], F32, tag="ot", name=f"otb{ch}_{rt}")
                for kt in range(nkt):
                    st_ps = ps_st.tile([128, 2 * CH], F32, tag="st", name=f"st{ch}_{rt}_{kt}")
                    for half, off in ((0, 0), (1, 64)):
                        nc.tensor.matmul(
                            st_ps[:, half * CH:(half + 1) * CH],
                            KT[rt][off:off + 64, kt * 128:(kt + 1) * 128],
                            QT[rt][off:off + 64, ch * CH:(ch + 1) * CH],
                            start=True, stop=True,
                        )
                    st_sb = stp.tile([128, 2 * CH], BF16, tag="stsb", name=f"se{ch}_{rt}_{kt}")
                    nc.scalar.activation(st_sb[:], st_ps[:], EXP, scale=0.125)
                    d = kt - 4 * ch
                    if d >= 0:
                        for half in range(2):
                            blk = st_sb[:, half * CH + d * 128: half * CH + (d + 1) * 128]
                            nc.vector.tensor_mul(blk, blk, mk_sb[:, 0:128])
                    # fill units between the score matmuls and the AV matmuls
                    while emitted * nsteps < (step + 1) * n_fill:
                        fq.popleft()()
                        emitted += 1
                    lo = max(d, 0) * 128 if d > 0 else 0
                    for half, ot_ps in ((0, ot_a), (1, ot_b)):
                        nc.tensor.matmul(
                            ot_ps[:, lo:],
                            V[kt][:, (rt * 2 + half) * (HD + 1):(rt * 2 + half + 1) * (HD + 1)],
                            st_sb[:, half * CH + lo:(half + 1) * CH],
                            start=(kt == 0), stop=(kt == nkt - 1),
                        )
                    step += 1
                normalize(ot_a, rt, 0, ch)
                normalize(ot_b, rt, 64, ch)
                if ch < NCH - 1:
                    emit_ag_in(ch, rt)
            if ch < NCH - 1:
                emit_a2a(ch)
                if ch > 0:
                    emit_otf(ch - 1)
                if ch > 1:
                    emit_cproj(ch - 2)
            else:
                # last chunk: otf(2) is free to read now (AG(2) done), then
                # the local-head partial c_proj + ReduceScatter
                emit_otf(ch - 1)
                for stl in range(4):
                    for n in range(2):
                        po = ps_mm.tile([128, CH], F32, tag="ps", name=f"p3_{stl}_{n}")
                        for k2 in range(NRT):
                            nc.tensor.matmul(
                                po[:],
                                OT[k2][:, ch * CH + stl * 128: ch * CH + (stl + 1) * 128],
                                wl_sb[:, k2 * D + n * CH: k2 * D + (n + 1) * CH],
                                start=(k2 == 0), stop=(k2 == NRT - 1),
                            )
                        pb = outp.tile([128, CH], BF16, tag="pb", name=f"pb{stl}_{n}")
                        if has_bp:
                            nc.vector.tensor_add(pb[:], po[:], bp3_sb[:, n * CH:(n + 1) * CH])
                        else:
                            # ACT is idle after the last exp; DVE is busy with
                            # the cproj casts — run the partial copies there
                            nc.scalar.copy(pb[:], po[:])
                        nc.sync.dma_start(
                            p3[stl * 128:(stl + 1) * 128, n * CH:(n + 1) * CH], pb[:]
                        )
                nc.gpsimd.collective_compute(
                    "ReduceScatter",
                    mybir.AluOpType.add,
                    replica_groups=[[0, 1, 2, 3], [4, 5, 6, 7]],
                    ins=[p3.opt()],
                    outs=[rs3.opt()],
                )
        emit_cproj(NCH - 3)
        emit_cproj(NCH - 2)
        rs3_sb = outp.tile([128, D], BF16, tag="r3sb", name="rs3_sb")
        nc.sync.dma_start(rs3_sb[:], rs3[:])
        nc.sync.dma_start(out3_d[:], rs3_sb[:])

    nc.compile()
    return nc


_prog_cache = {}


def _get_prog(has_bv, has_bp, has_bqk):
    key = (has_bv, has_bp, has_bqk)
    if key not in _prog_cache:
        _prog_cache[key] = _build(*key)
    return _prog_cache[key]


def _pack_kmajor(w):
    # [D, X] -> [128, NK*X] with col = kt*X + x, so sbuf[:, kt*X+x] = w[kt*128+p, x]
    dd, x = w.shape
    return np.ascontiguousarray(
        w.reshape(NK, 128, x).transpose(1, 0, 2).reshape(128, NK * x)
    )


def _prepare(x, w_attn, b_attn, w_proj, b_proj):
    x = np.asarray(x, dtype=np.float32)
    w_attn = np.asarray(w_attn, dtype=np.float32)
    b_attn = np.asarray(b_attn, dtype=np.float32)
    w_proj = np.asarray(w_proj, dtype=np.float32)
    b_proj = np.asarray(b_proj, dtype=np.float32)

    has_bv = bool(np.any(b_attn[2 * D:]))
    has_bp = bool(np.any(b_proj))
    has_bqk = bool(np.any(b_attn[:2 * D]))
    nc = _get_prog(has_bv, has_bp, has_bqk)

    ii = np.arange(128)[:, None]
    jj = np.arange(128)[None, :]
    masks = (jj >= ii).astype(np.float32).astype(ml_dtypes.bfloat16)

    # per-batch activations, transposed + packed to the SBUF layout
    # (col = sq*4096 + kt*512 + t), shared across the 4 group-cores
    xT_b = [
        np.ascontiguousarray(
            x[b].reshape(NCH, CH, NK, 128).transpose(3, 0, 2, 1).reshape(128, NK * S)
        ).astype(ml_dtypes.bfloat16)
        for b in range(B)
    ]
    # per-group packed weights (shared across the 2 batch-cores)
    wq_g, wk_g, wv_g, bq_g, bk_g, bv_g = [], [], [], [], [], []
    for g in range(HG):
        q0 = g * DG
        k0 = D + g * DG
        v0 = 2 * D + g * DG
        wv_ext = np.zeros((D, VW), dtype=np.float32)
        for hl in range(HG):
            wv_ext[:, hl * (HD + 1):hl * (HD + 1) + HD] = \
                w_attn[:, v0 + hl * HD: v0 + (hl + 1) * HD]
        wqkv = np.concatenate(
            [
                _pack_kmajor(w_attn[:, q0:q0 + DG]),
                _pack_kmajor(w_attn[:, k0:k0 + DG]),
                _pack_kmajor(wv_ext),
                (jj >= ii).astype(np.float32),
            ],
            axis=1,
        )
        wq_g.append(np.ascontiguousarray(wqkv).astype(ml_dtypes.bfloat16))
        bq_g.append(np.ascontiguousarray(b_attn[q0:q0 + DG, None]))
        bk_g.append(np.ascontiguousarray(b_attn[k0:k0 + DG, None]))
        bv_g.append(np.ascontiguousarray(b_attn[v0:v0 + DG, None]))
    wp_pack = _pack_kmajor(w_proj).astype(ml_dtypes.bfloat16)
    bp_tile = np.broadcast_to(b_proj, (128, D)).astype(np.float32)
    bp_zero = np.zeros((128, D), dtype=np.float32)
    # per-group local wp rows [g*256, (g+1)*256) packed k2-major
    wl_g = [
        np.ascontiguousarray(
            w_proj[g * DG:(g + 1) * DG, :].reshape(NRT, 128, D)
            .transpose(1, 0, 2).reshape(128, NRT * D)
        ).astype(ml_dtypes.bfloat16)
        for g in range(HG)
    ]

    in_maps = []
    for c in range(N_CORES):
        b, g = divmod(c, 4)
        in_maps.append({
            "xT": xT_b[b],
            "wqkv": wq_g[g],
            "wp": wp_pack,
            "wploc": wl_g[g],
            "bq": bq_g[g],
            "bk": bk_g[g],
            "bv": bv_g[g],
            "bp": bp_tile,
            "bp3": bp_tile if g == 0 else bp_zero,
            "masks": masks,
            "gcol": np.array([[g * 128]], dtype=np.uint32),
        })
    return nc, in_maps


def _assemble(results):
    out = np.empty((B, S, D), dtype=np.float32)
    for c in range(N_CORES):
        b, g = divmod(c, 4)
        o = results[c]["out"]
        for ch in range(NCH - 1):
            tok = ch * CH + g * 128
            out[b, tok:tok + 128, :] = o[ch * 128:(ch + 1) * 128, :]
        tok = (NCH - 1) * CH + g * 128
        out[b, tok:tok + 128, :] = results[c]["out3"].astype(np.float32)
    return out


def kernel(x, w_attn, b_attn, w_proj, b_proj):
    nc, in_maps = _prepare(x, w_attn, b_attn, w_proj, b_proj)
    res = run_bass_kernel_spmd(nc, in_maps, list(range(N_CORES)))
    return _assemble(res.results)


# revision 52
# speedup vs baseline: 1.0452x; 1.0452x over previous
"""GPT2 eager causal attention (B=2, S=2048, D=1024, H=16, HD=64) on 8 TRN2 NeuronCores.

Sharding (data + head/tensor parallel): core c -> (batch b = c//4, head-group
g = c%4), 4 heads per group.  Token ownership for the output: core (b, g) owns
token rows [ch*512 + g*128, +128) of batch b, for each 512-token chunk ch.

v2 pipeline (vs the RS baseline):
  - x is transposed on the HOST -> xT [D, S]; no transpose-DMAs on device.
  - weights are host-packed into SBUF layout -> one large DMA each, spread
    across the SP/ACT/gpsimd DMA queues so nothing serializes at startup.
  - QT/KT [256, S] and V [S, 260] (ones-column per head for the free softmax
    denominator) as in the baseline, but QKV compute for token-chunk sq>=1 is
    software-pipelined INTO the attention loop of chunk sq-1 to keep PE dense
    (HAM stays warm) and to overlap ACT-exp with PE matmuls.
  - scores: the two heads of a KT row-tile run as CONCURRENT 64-contract
    matmuls on PE row-groups 0-63/64-127 writing adjacent PSUM banks; one
    batched EXP (FD=1024) covers both.  Diagonal tiles exp the full tile
    (garbage prefix cols are simply never streamed by the AV matmul) and
    triangular-mask only the 128-wide diagonal block.
  - softmax normalize: DVE reciprocal of the ones-row directly from PSUM,
    gpsimd partition_broadcast (instead of a PE rank-1 matmul), DVE multiply.
  - c_proj: per 512-token chunk, AllToAll over the quad exchanges bf16 head
    outputs so each core receives ALL 1024 channels for ITS OWN 128-token
    slice; c_proj then contracts the full D with the full w_proj.  ~4x less
    wire than the fp32 ReduceScatter and a much shorter serial tail.
"""
from collections import deque
from contextlib import ExitStack

import ml_dtypes
import numpy as np

import concourse.bacc as bacc
import concourse.mybir as mybir
import concourse.tile as tile
from concourse.bass import ds as bass_ds
from concourse.bass_utils import run_bass_kernel_spmd

F32 = mybir.dt.float32
BF16 = mybir.dt.bfloat16
U16 = mybir.dt.uint16

B, S, D, H, HD = 2, 2048, 1024, 16, 64
N_CORES = 8
HG = 4               # heads per group
DG = HG * HD         # 256 q/k channels per group
VW = HG * (HD + 1)   # 260: 64 v-cols + 1 ones-col per head
NK = D // 128        # 8 contraction tiles over d
NS = S // 128        # 16 token tiles
CH = 512             # q-chunk (one PSUM bank of fp32)
NCH = S // CH        # 4
NRT = DG // 128      # 2 channel row-tiles (head pairs) per group
SQW = NK * CH        # 4096: xT sbuf columns per token chunk

EXP = mybir.ActivationFunctionType.Exp
IDENT = mybir.ActivationFunctionType.Identity


def _build(has_bv: bool, has_bp: bool, has_bqk: bool):
    nc = bacc.Bacc("TRN2", target_bir_lowering=False, debug=False, num_devices=N_CORES)

    xT_d = nc.dram_tensor("xT", [128, NK * S], BF16, kind="ExternalInput").ap()
    # wq | wk | wv | mask packed back-to-back: one startup DMA
    WQKV = 2 * NK * DG + NK * VW + 128
    wqkv_d = nc.dram_tensor("wqkv", [128, WQKV], BF16, kind="ExternalInput").ap()
    wp_d = nc.dram_tensor("wp", [128, NK * D], BF16, kind="ExternalInput").ap()
    bq_d = nc.dram_tensor("bq", [DG, 1], F32, kind="ExternalInput").ap()
    bk_d = nc.dram_tensor("bk", [DG, 1], F32, kind="ExternalInput").ap()
    bv_d = nc.dram_tensor("bv", [DG, 1], F32, kind="ExternalInput").ap()
    bp_d = nc.dram_tensor("bp", [128, D], F32, kind="ExternalInput").ap()
    mk_d = nc.dram_tensor("masks", [128, 128], BF16, kind="ExternalInput").ap()
    gc_d = nc.dram_tensor("gcol", [1, 1], mybir.dt.uint32, kind="ExternalInput").ap()
    out_d = nc.dram_tensor("out", [NCH * 128, D], F32, kind="ExternalOutput").ap()

    with ExitStack() as ctx:
        tc = ctx.enter_context(tile.TileContext(nc))
        wpool = ctx.enter_context(tc.tile_pool(name="w", bufs=1))
        qkvp = ctx.enter_context(tc.tile_pool(name="qkv", bufs=1))
        stp = ctx.enter_context(tc.tile_pool(name="stx", bufs=3))
        nrm = ctx.enter_context(tc.tile_pool(name="nrm", bufs=3))
        otfp = ctx.enter_context(tc.tile_pool(name="otf", bufs=2))
        outp = ctx.enter_context(tc.tile_pool(name="outp", bufs=3))
        ps_st = ctx.enter_context(tc.tile_pool(name="psst", bufs=2, space="PSUM"))
        ps_ot = ctx.enter_context(tc.tile_pool(name="psot", bufs=2, space="PSUM"))
        ps_mm = ctx.enter_context(tc.tile_pool(name="psmm", bufs=2, space="PSUM"))
        dram = ctx.enter_context(tc.tile_pool(name="dram", bufs=1, space="DRAM"))

        # ---- tiny exp at t=0 forces the ACT table load to overlap the DMAs
        dmy = wpool.tile([128, 1], F32, tag="dmy", name="dmy")
        dmo = wpool.tile([128, 1], F32, tag="dmo", name="dmo")
        nc.vector.memset(dmy[:], 0.0)
        nc.scalar.activation(dmo[:], dmy[:], EXP, scale=1.0)

        # ---- weights: wq|wk|wv|mask as ONE contiguous DMA on the ACT queue,
        # wp (only needed ~100us in) on the gpsimd queue
        wqkv_sb = wpool.tile([128, WQKV], BF16, tag="wqkv", name="wqkv_sb")
        wp_sb = wpool.tile([128, NK * D], BF16, tag="wp", name="wp_sb")
        nc.scalar.dma_start(wqkv_sb[:], wqkv_d[:])
        nc.gpsimd.dma_start(wp_sb[:], wp_d[:])
        wq_sb = wqkv_sb[:, 0:NK * DG]
        wk_sb = wqkv_sb[:, NK * DG:2 * NK * DG]
        wv_sb = wqkv_sb[:, 2 * NK * DG:2 * NK * DG + NK * VW]
        mk_sb = wqkv_sb[:, 2 * NK * DG + NK * VW:WQKV]
        bq_sb = bk_sb = bv_sb = bp_sb = None
        if has_bqk:
            bq_sb = wpool.tile([128, NRT], F32, tag="bq", name="bq_sb")
            bk_sb = wpool.tile([128, NRT], F32, tag="bk", name="bk_sb")
            for rt in range(NRT):
                nc.scalar.dma_start(bq_sb[:, rt:rt + 1], bq_d[rt * 128:(rt + 1) * 128, :])
                nc.scalar.dma_start(bk_sb[:, rt:rt + 1], bk_d[rt * 128:(rt + 1) * 128, :])
        if has_bv:
            bv_sb = wpool.tile([128, NRT], F32, tag="bv", name="bv_sb")
            for rt in range(NRT):
                nc.scalar.dma_start(bv_sb[:, rt:rt + 1], bv_d[rt * 128:(rt + 1) * 128, :])
        if has_bp:
            bp_sb = wpool.tile([128, D], F32, tag="bp", name="bp_sb")
            nc.scalar.dma_start(bp_sb[:], bp_d[:])

        # ---- V tiles exist up-front; set the whole tile to bf16 1.0 now so
        # the later per-head copies only write the 64 v-columns
        V = []
        for st in range(NS):
            vt = qkvp.tile([128, VW], BF16, tag=f"v{st}", name=f"v{st}")
            nc.vector.memset(vt[:].bitcast(U16), 0x3F80)
            V.append(vt)

        # ---- xT loads: host pre-packed to SBUF layout (col = sq*4096 + kt*512
        # + t), so each chunk is one fully-contiguous 1MB DMA
        xT = qkvp.tile([128, NK * S], BF16, tag="xT", name="xT")
        # sq0 lands in two halves so the first Q chain starts ~2us sooner
        nc.sync.dma_start(xT[:, 0:SQW // 2], xT_d[:, 0:SQW // 2])
        nc.sync.dma_start(xT[:, SQW // 2:SQW], xT_d[:, SQW // 2:SQW])
        for sq in range(1, NCH):
            nc.sync.dma_start(
                xT[:, sq * SQW:(sq + 1) * SQW], xT_d[:, sq * SQW:(sq + 1) * SQW]
            )

        QT = [qkvp.tile([128, S], BF16, tag=f"qt{rt}", name=f"qt{rt}") for rt in range(NRT)]
        KT = [qkvp.tile([128, S], BF16, tag=f"kt{rt}", name=f"kt{rt}") for rt in range(NRT)]
        OT = [qkvp.tile([128, S], BF16, tag=f"ot{rt}", name=f"ot{rt}") for rt in range(NRT)]

        def emit_qk(store, w_sb, b_sb, rt, sq):
            ps = ps_mm.tile([128, CH], F32, tag="ps", name=f"psqk{rt}_{sq}")
            for kt in range(NK):
                nc.tensor.matmul(
                    ps[:],
                    w_sb[:, kt * DG + rt * 128: kt * DG + (rt + 1) * 128],
                    xT[:, sq * SQW + kt * CH: sq * SQW + (kt + 1) * CH],
                    start=(kt == 0), stop=(kt == NK - 1),
                )
            dst = store[rt][:, sq * CH:(sq + 1) * CH]
            if b_sb is not None:
                nc.scalar.activation(dst, ps[:], IDENT, bias=b_sb[:, rt:rt + 1])
            else:
                nc.vector.tensor_copy(dst, ps[:])

        def emit_v(st):
            sq, j = divmod(st, 4)
            ps = ps_mm.tile([128, CH], F32, tag="ps", name=f"psv{st}")
            for kt in range(NK):
                nc.tensor.matmul(
                    ps[:, :VW],
                    xT[:, sq * SQW + kt * CH + j * 128: sq * SQW + kt * CH + (j + 1) * 128],
                    wv_sb[:, kt * VW:(kt + 1) * VW],
                    start=(kt == 0), stop=(kt == NK - 1),
                )
            for hl in range(HG):
                nc.vector.tensor_copy(
                    V[st][:, hl * (HD + 1): hl * (HD + 1) + HD],
                    ps[:, hl * (HD + 1): hl * (HD + 1) + HD],
                )

        # ---- QKV for chunk 0 up-front; chunks 1-3 become fill units that are
        # interleaved into the attention loop (PE never idles on exp waits)
        for rt in range(NRT):
            emit_qk(QT, wq_sb, bq_sb, rt, 0)
            emit_qk(KT, wk_sb, bk_sb, rt, 0)
        for st in range(4):
            emit_v(st)

        fills = {ch: deque() for ch in range(NCH)}
        for sq in range(1, NCH):
            fq = fills[sq - 1]
            fq.append(lambda sq=sq: emit_qk(QT, wq_sb, bq_sb, 0, sq))
            fq.append(lambda sq=sq: emit_qk(KT, wk_sb, bk_sb, 0, sq))
            fq.append(lambda sq=sq: emit_v(sq * 4 + 0))
            fq.append(lambda sq=sq: emit_qk(QT, wq_sb, bq_sb, 1, sq))
            fq.append(lambda sq=sq: emit_qk(KT, wk_sb, bk_sb, 1, sq))
            fq.append(lambda sq=sq: emit_v(sq * 4 + 1))
            fq.append(lambda sq=sq: emit_v(sq * 4 + 2))
            fq.append(lambda sq=sq: emit_v(sq * 4 + 3))

        # ---- collective buffers: per 512-token chunk, AllGather the quad's
        # bf16 head outputs [256 ch, 512 tok] -> [1024 ch, 512 tok]; each core
        # then reads back only ITS OWN 128-token column slice (dynamic offset
        # g*128 from the per-core "gcol" input).
        ag_in = [dram.tile([DG, CH], BF16, tag=f"agi{c}", name=f"ag_in{c}")
                 for c in range(NCH - 1)]
        ag_out = [dram.tile([HG * DG, CH], BF16, tag=f"ago{c}", name=f"ag_out{c}")
                  for c in range(NCH - 1)]
        # chunk 3's AllGather is split by head-pair: pair A's 128KB gather
        # launches ~13us before pair B's attention finishes, so only the
        # second (small) gather is exposed on the tail
        ag3_in = [dram.tile([128, CH], BF16, tag=f"a3i{r}", name=f"ag3_in{r}")
                  for r in range(NRT)]
        ag3_out = [dram.tile([HG * 128, CH], BF16, tag=f"a3o{r}", name=f"ag3_out{r}")
                   for r in range(NRT)]

        # gc load rides behind the xT loads — it's only needed ~100us in
        gc_sb = wpool.tile([1, 1], mybir.dt.uint32, tag="gc", name="gc_sb")
        nc.scalar.dma_start(gc_sb[:], gc_d[:])

        def emit_ag_in(ch, rt):
            nc.sync.dma_start(
                ag_in[ch][rt * 128:(rt + 1) * 128, :],
                OT[rt][:, ch * CH:(ch + 1) * CH],
            )

        def emit_a2a(ch):
            nc.gpsimd.collective_compute(
                "AllGather",
                mybir.AluOpType.bypass,
                replica_groups=[[0, 1, 2, 3], [4, 5, 6, 7]],
                ins=[ag_in[ch].opt()],
                outs=[ag_out[ch].opt()],
            )

        otf_tiles = {}

        # otf readback rides the gpsimd (SWDGE) queue: emitted right after the
        # NEXT chunk's AG trigger, its wait on AG(ch)-done never heads-of-line
        # blocks the SP queue or the broadcasts
        gcol_gp = None

        def emit_otf(ch):
            nonlocal gcol_gp
            if gcol_gp is None:
                _r = nc.gpsimd.alloc_register("gcol_gp_reg")
                nc.gpsimd.reg_load(_r, gc_sb[0:1, 0:1])
                gcol_gp = nc.gpsimd.snap(_r, donate=True, min_val=0, max_val=(HG - 1) * 128)
            otf = otfp.tile([128, NK * 128], BF16, tag="otf", name=f"otf{ch}")
            src = ag_out[ch][:, bass_ds(gcol_gp, 128)]
            nc.gpsimd.dma_start(
                otf[:].rearrange("p (kt t) -> p kt t", kt=NK),
                src.rearrange("(kt p) t -> p kt t", kt=NK),
            )
            otf_tiles[ch] = otf

        def emit_cproj(ch):
            otf = otf_tiles[ch]
            for n in range(2):
                po = ps_mm.tile([128, CH], F32, tag="ps", name=f"po{ch}_{n}")
                for kt in range(NK):
                    nc.tensor.matmul(
                        po[:],
                        otf[:, kt * 128:(kt + 1) * 128],
                        wp_sb[:, kt * D + n * CH: kt * D + (n + 1) * CH],
                        start=(kt == 0), stop=(kt == NK - 1),
                    )
                ob = outp.tile([128, CH], F32, tag="ob", name=f"ob{ch}_{n}")
                if has_bp:
                    nc.vector.tensor_add(ob[:], po[:], bp_sb[:, n * CH:(n + 1) * CH])
                else:
                    nc.vector.tensor_copy(ob[:], po[:])
                nc.sync.dma_start(out_d[ch * 128:(ch + 1) * 128, n * CH:(n + 1) * CH], ob[:])

        def normalize(ot_ps, rt, off, ch):
            den = nrm.tile([1, CH], F32, tag="den", name=f"den{ch}_{rt}_{off}")
            nc.vector.tensor_copy(den[:], ot_ps[64:65, :])
            rden = nrm.tile([1, CH], F32, tag="rden", name=f"rden{ch}_{rt}_{off}")
            nc.vector.reciprocal_approx_fast(rden[:], den[:])
            rbc = nrm.tile([64, CH], F32, tag="rbc", name=f"rbc{ch}_{rt}_{off}")
            nc.gpsimd.partition_broadcast(rbc[:], rden[:], channels=64)
            dst = OT[rt][off:off + 64, ch * CH:(ch + 1) * CH]
            nc.vector.tensor_mul(dst, ot_ps[0:64, :], rbc[:])
            if has_bv:
                nc.vector.tensor_scalar_add(dst, dst, bv_sb[off:off + 64, rt:rt + 1])

        # ---- attention chunks, with QKV fill units woven in.  cproj for
        # chunk ch-1 (otf readback + matmuls) is emitted right after chunk
        # ch's AllGather trigger: by then AG(ch-1) has completed, so neither
        # the PE FIFO nor the SP DMA queue ever blocks on a collective
        # mid-stream — only the final chunk's AG is exposed.
        for ch in range(NCH):
            nkt = 4 * (ch + 1)
            nsteps = 2 * nkt
            fq = fills[ch]
            n_fill = len(fq)
            emitted = 0
            step = 0
            for rt in range(NRT):
                ot_a = ps_ot.tile([65, CH], F32, tag="ot", name=f"ota{ch}_{rt}")
                ot_b = ps_ot.tile([65, CH], F32, tag="ot", name=f"otb{ch}_{rt}")
                for kt in range(nkt):
                    st_ps = ps_st.tile([128, 2 * CH], F32, tag="st", name=f"st{ch}_{rt}_{kt}")
                    for half, off in ((0, 0), (1, 64)):
                        nc.tensor.matmul(
                            st_ps[:, half * CH:(half + 1) * CH],
                            KT[rt][off:off + 64, kt * 128:(kt + 1) * 128],
                            QT[rt][off:off + 64, ch * CH:(ch + 1) * CH],
                            start=True, stop=True,
                        )
                    st_sb = stp.tile([128, 2 * CH], BF16, tag="stsb", name=f"se{ch}_{rt}_{kt}")
                    nc.scalar.activation(st_sb[:], st_ps[:], EXP, scale=0.125)
                    d = kt - 4 * ch
                    if d >= 0:
                        for half in range(2):
                            blk = st_sb[:, half * CH + d * 128: half * CH + (d + 1) * 128]
                            nc.vector.tensor_mul(blk, blk, mk_sb[:, 0:128])
                    # fill units between the score matmuls and the AV matmuls
                    while emitted * nsteps < (step + 1) * n_fill:
                        fq.popleft()()
                        emitted += 1
                    lo = max(d, 0) * 128 if d > 0 else 0
                    for half, ot_ps in ((0, ot_a), (1, ot_b)):
                        nc.tensor.matmul(
                            ot_ps[:, lo:],
                            V[kt][:, (rt * 2 + half) * (HD + 1):(rt * 2 + half + 1) * (HD + 1)],
                            st_sb[:, half * CH + lo:(half + 1) * CH],
                            start=(kt == 0), stop=(kt == nkt - 1),
                        )
                    step += 1
                normalize(ot_a, rt, 0, ch)
                normalize(ot_b, rt, 64, ch)
                if ch < NCH - 1:
                    emit_ag_in(ch, rt)
                else:
                    nc.sync.dma_start(ag3_in[rt][:], OT[rt][:, ch * CH:(ch + 1) * CH])
                    nc.gpsimd.collective_compute(
                        "AllGather",
                        mybir.AluOpType.bypass,
                        replica_groups=[[0, 1, 2, 3], [4, 5, 6, 7]],
                        ins=[ag3_in[rt].opt()],
                        outs=[ag3_out[rt].opt()],
                    )
            if ch < NCH - 1:
                emit_a2a(ch)
                if ch > 0:
                    emit_otf(ch - 1)
                if ch > 1:
                    emit_cproj(ch - 2)
            else:
                # otf(2) is free to read now (AG(2) long done), then the two
                # half-readbacks of chunk 3's gathered head outputs
                emit_otf(ch - 1)
                otf3 = otfp.tile([128, NK * 128], BF16, tag="otf", name="otf3")
                for rt in range(NRT):
                    src = ag3_out[rt][:, bass_ds(gcol_gp, 128)].rearrange(
                        "(o g p) t -> p o g t", o=1, g=HG
                    )
                    dst = otf3[:].rearrange(
                        "p (g e t) -> p e g t", e=NRT, g=HG
                    )[:, rt:rt + 1]
                    nc.gpsimd.dma_start(dst, src)
        emit_cproj(NCH - 3)
        emit_cproj(NCH - 2)
        # cproj(3): contract the gathered halves in arrival order (pair A's
        # kt blocks while pair B's gather is still in flight)
        for n in range(2):
            po = ps_mm.tile([128, CH], F32, tag="ps", name=f"po3_{n}")
            first = True
            for rt in range(NRT):
                for g in range(HG):
                    kt = 2 * g + rt
                    nc.tensor.matmul(
                        po[:],
                        otf3[:, kt * 128:(kt + 1) * 128],
                        wp_sb[:, kt * D + n * CH: kt * D + (n + 1) * CH],
                        start=first, stop=(rt == NRT - 1 and g == HG - 1),
                    )
                    first = False
            ob = outp.tile([128, CH], F32, tag="ob", name=f"ob3_{n}")
            if has_bp:
                nc.vector.tensor_add(ob[:], po[:], bp_sb[:, n * CH:(n + 1) * CH])
            else:
                nc.vector.tensor_copy(ob[:], po[:])
            nc.sync.dma_start(
                out_d[(NCH - 1) * 128:NCH * 128, n * CH:(n + 1) * CH], ob[:]
            )

    nc.compile()
    return nc


_prog_cache = {}


def _get_prog(has_bv, has_bp, has_bqk):
    key = (has_bv, has_bp, has_bqk)
    if key not in _prog_cache:
        _prog_cache[key] = _build(*key)
    return _prog_cache[key]


def _pack_kmajor(w):
    # [D, X] -> [128, NK*X] with col = kt*X + x, so sbuf[:, kt*X+x] = w[kt*128+p, x]
    dd, x = w.shape
    return np.ascontiguousarray(
        w.reshape(NK, 128, x).transpose(1, 0, 2).reshape(128, NK * x)
    )


def _prepare(x, w_attn, b_attn, w_proj, b_proj):
    x = np.asarray(x, dtype=np.float32)
    w_attn = np.asarray(w_attn, dtype=np.float32)
    b_attn = np.asarray(b_attn, dtype=np.float32)
    w_proj = np.asarray(w_proj, dtype=np.float32)
    b_proj = np.asarray(b_proj, dtype=np.float32)

    has_bv = bool(np.any(b_attn[2 * D:]))
    has_bp = bool(np.any(b_proj))
    has_bqk = bool(np.any(b_attn[:2 * D]))
    nc = _get_prog(has_bv, has_bp, has_bqk)

    ii = np.arange(128)[:, None]
    jj = np.arange(128)[None, :]
    masks = (jj >= ii).astype(np.float32).astype(ml_dtypes.bfloat16)

    # per-batch activations, transposed + packed to the SBUF layout
    # (col = sq*4096 + kt*512 + t), shared across the 4 group-cores
    xT_b = [
        np.ascontiguousarray(
            x[b].reshape(NCH, CH, NK, 128).transpose(3, 0, 2, 1).reshape(128, NK * S)
        ).astype(ml_dtypes.bfloat16)
        for b in range(B)
    ]
    # per-group packed weights (shared across the 2 batch-cores)
    wq_g, wk_g, wv_g, bq_g, bk_g, bv_g = [], [], [], [], [], []
    for g in range(HG):
        q0 = g * DG
        k0 = D + g * DG
        v0 = 2 * D + g * DG
        wv_ext = np.zeros((D, VW), dtype=np.float32)
        for hl in range(HG):
            wv_ext[:, hl * (HD + 1):hl * (HD + 1) + HD] = \
                w_attn[:, v0 + hl * HD: v0 + (hl + 1) * HD]
        wqkv = np.concatenate(
            [
                _pack_kmajor(w_attn[:, q0:q0 + DG]),
                _pack_kmajor(w_attn[:, k0:k0 + DG]),
                _pack_kmajor(wv_ext),
                (jj >= ii).astype(np.float32),
            ],
            axis=1,
        )
        wq_g.append(np.ascontiguousarray(wqkv).astype(ml_dtypes.bfloat16))
        bq_g.append(np.ascontiguousarray(b_attn[q0:q0 + DG, None]))
        bk_g.append(np.ascontiguousarray(b_attn[k0:k0 + DG, None]))
        bv_g.append(np.ascontiguousarray(b_attn[v0:v0 + DG, None]))
    wp_pack = _pack_kmajor(w_proj).astype(ml_dtypes.bfloat16)
    bp_tile = np.broadcast_to(b_proj, (128, D)).astype(np.float32)

    in_maps = []
    for c in range(N_CORES):
        b, g = divmod(c, 4)
        in_maps.append({
            "xT": xT_b[b],
            "wqkv": wq_g[g],
            "wp": wp_pack,
            "bq": bq_g[g],
            "bk": bk_g[g],
            "bv": bv_g[g],
            "bp": bp_tile,
            "masks": masks,
            "gcol": np.array([[g * 128]], dtype=np.uint32),
        })
    return nc, in_maps


def _assemble(results):
    out = np.empty((B, S, D), dtype=np.float32)
    for c in range(N_CORES):
        b, g = divmod(c, 4)
        o = results[c]["out"]
        for ch in range(NCH):
            tok = ch * CH + g * 128
            out[b, tok:tok + 128, :] = o[ch * 128:(ch + 1) * 128, :]
    return out


def kernel(x, w_attn, b_attn, w_proj, b_proj):
    nc, in_maps = _prepare(x, w_attn, b_attn, w_proj, b_proj)
    res = run_bass_kernel_spmd(nc, in_maps, list(range(N_CORES)))
    return _assemble(res.results)
